# revision 1
# baseline (speedup 1.0000x reference)
"""Self-contained Trainium2 Bass kernel for nn_Attention_37125697306831.

Multi-head attention block: B=4, H=W=48 (N=2304), C=256, 8 heads, head_dim=32,
RoPE (rotate-half), softmax attention, separate Q/K/V projections (K without
bias), output projection with bias.

Sharding: 8 cores = (batch b in 0..3) x (query half in 0..1). Each core:
  - computes Q for its 1152 queries (all heads), K/V for all 2304 keys of its
    batch, attention + output projection for its 1152 query rows.
  - no collectives; output rows are disjoint across cores.

On-chip layouts:
  - xT [ci, n], qT/kT [c, n] (head dim on partitions), V natural [n, c].
  - scores computed transposed S.T[m keys, n queries] via row-packed K=32
    fp16 matmuls (tile_position), exp on ScalarE PSUM->SBUF (bottleneck
    engine), A@V as col-packed fp16 matmuls contracting over keys (K=128),
    softmax sums via ones-column matmuls, normalization via per-head K=1
    broadcast matmuls + DVE multiply, output projection consumes normalized
    out.T as lhsT giving y [n, co] for contiguous DMA out.

All matmul operands are fp16 (PE full rate; PSUM accumulation is fp32);
elementwise math (RoPE, exp, reciprocal, bias adds) stays fp32.
"""

import numpy as np
from contextlib import ExitStack

import concourse.bass as bass
import concourse.tile as tile
from concourse import bacc, mybir
from concourse.bass_utils import run_bass_kernel_spmd

F32 = mybir.dt.float32
F16 = mybir.dt.float16
AF = mybir.ActivationFunctionType

B, HH, WW, C = 4, 48, 48, 256
N = HH * WW            # 2304 keys per batch
NQ = N // 2            # 1152 queries per core
NH, HD, D2 = 8, 32, 16
NT = N // 128          # 18 key m-tiles
ROPE_BASE = 10000.0
SCALE = HD ** -0.5

QCH = [(0, 512), (512, 512), (1024, 128)]                       # query chunks
KCH = [(0, 512), (512, 512), (1024, 512), (1536, 512), (2048, 256)]
GROUPS = [(0, 3), (3, 3), (6, 2)]                               # (head0, size)

# DRAM input dtypes: fp16 for matmul operands, fp32 for DVE-side constants
IN_SPECS = [
    ("xT", [C, N], F16), ("xTq", [C, NQ], F16),
    ("wqT", [C, C], F16), ("wqrT", [C, C], F16),
    ("wkT", [C, C], F16), ("wkrT", [C, C], F16),
    ("wvT", [C, C], F16), ("woT", [C, C], F16),
    ("qb", [C, 1], F32), ("rqb", [C, 1], F32), ("vb", [1, C], F16),
    ("ones", [128, 128], F16),
    ("bob", [128, C], F32),
    ("CTQ", [C, NQ], F16), ("STQ", [C, NQ], F16),
    ("CTK", [C, N], F16), ("STK", [C, N], F16),
]

NG = 4                  # 4 groups of 2 heads
VW = 33                 # V columns per head incl. the ones column
mul = mybir.AluOpType.mult
add_op = mybir.AluOpType.add


def emit(tc, io, R=1, ablate=()):
    nc = tc.nc
    ctx = ExitStack()
    with ctx:
        consts = ctx.enter_context(tc.tile_pool(name="consts", bufs=1))
        sb = ctx.enter_context(tc.tile_pool(name="sb", bufs=1))
        tmp = ctx.enter_context(tc.tile_pool(name="tmp", bufs=4))
        ptpool = ctx.enter_context(tc.tile_pool(name="pt", bufs=3))
        outpool = ctx.enter_context(tc.tile_pool(name="outT", bufs=2))
        ypool = ctx.enter_context(tc.tile_pool(name="y", bufs=3))
        rpool = ctx.enter_context(tc.tile_pool(name="recip", bufs=2))
        # PSUM: scores 3x2 banks + av 2x1 = 8 banks; rf/y borrow scp slots
        scp = ctx.enter_context(tc.tile_pool(name="scp", bufs=3, space="PSUM"))
        avp = ctx.enter_context(tc.tile_pool(name="avp", bufs=2, space="PSUM"))

        dtypes = {name: dt for name, _, dt in IN_SPECS}

        def load(name, shape):
            t = consts.tile(shape, dtypes[name], tag=name)
            nc.sync.dma_start(t[:], io[name][:])
            return t

        # ---- constant loads ----------------------------------------------
        xT0 = consts.tile([128, N], F16, tag="xT0")
        nc.sync.dma_start(xT0[:], io["xT"][0:128, :])
        xT1 = consts.tile([128, N], F16, tag="xT1")
        nc.sync.dma_start(xT1[:], io["xT"][128:256, :])
        xTq0 = consts.tile([128, NQ], F16, tag="xTq0")
        nc.sync.dma_start(xTq0[:], io["xTq"][0:128, :])
        xTq1 = consts.tile([128, NQ], F16, tag="xTq1")
        nc.sync.dma_start(xTq1[:], io["xTq"][128:256, :])

        def load2(name, free):
            a = consts.tile([128, free], dtypes[name], tag=name + "0")
            nc.sync.dma_start(a[:], io[name][0:128, :])
            b_ = consts.tile([128, free], dtypes[name], tag=name + "1")
            nc.sync.dma_start(b_[:], io[name][128:256, :])
            return [a, b_]

        wq = load2("wqT", C)
        wqr = load2("wqrT", C)
        wk = load2("wkT", C)
        wkr = load2("wkrT", C)
        wv = load2("wvT", C)
        wo = load2("woT", C)
        qb = load2("qb", 1)
        rqb = load2("rqb", 1)
        vb = load("vb", [1, C])
        ones = load("ones", [128, 128])
        bob = load("bob", [128, C])
        ctq = load2("CTQ", NQ)
        stq = load2("STQ", NQ)
        ctk = load2("CTK", N)
        stk = load2("STK", N)

        if R > 1:
            loop_ctx = tc.For_i(0, R, 1)
            loop_ctx.__enter__()

        # ---- phase 1: projections + RoPE ---------------------------------
        qT = [sb.tile([128, NQ], F16, tag=f"qT{i}", name=f"qT{i}") for i in range(2)]
        kT = [sb.tile([128, N], F16, tag=f"kT{i}", name=f"kT{i}") for i in range(2)]
        vsb = sb.tile([128, NT * NH * VW], F16, tag="v")
        nc.gpsimd.memset(vsb[:], 1.0)

        def proj_rope(dst, w, wr, bias, rbias, xa, xb, ct, st, off, cw):
            # dst[:, off:off+cw] = (w.T x + b) * ct + (wr.T x + rb) * st
            for cg in range(2):
                ps = scp.tile([128, 2, 512], F32, tag="sc")
                nc.tensor.matmul(ps[:, 0, :cw], w[0][:, bass.ts(cg, 128)],
                                 xa[:, off:off + cw], start=True, stop=False)
                nc.tensor.matmul(ps[:, 0, :cw], w[1][:, bass.ts(cg, 128)],
                                 xb[:, off:off + cw], start=False, stop=True)
                nc.tensor.matmul(ps[:, 1, :cw], wr[0][:, bass.ts(cg, 128)],
                                 xa[:, off:off + cw], start=True, stop=False)
                nc.tensor.matmul(ps[:, 1, :cw], wr[1][:, bass.ts(cg, 128)],
                                 xb[:, off:off + cw], start=False, stop=True)
                b0 = bias[cg] if isinstance(bias[cg], float) else bias[cg][:]
                b1 = rbias[cg] if isinstance(rbias[cg], float) else rbias[cg][:]
                t1 = tmp.tile([128, 512], F32, tag="t1")
                nc.vector.scalar_tensor_tensor(
                    t1[:, 0:cw], ps[:, 0, 0:cw], b0,
                    ct[cg][:, off:off + cw], op0=add_op, op1=mul)
                t2 = tmp.tile([128, 512], F32, tag="t2")
                nc.vector.scalar_tensor_tensor(
                    t2[:, 0:cw], ps[:, 1, 0:cw], b1,
                    st[cg][:, off:off + cw], op0=add_op, op1=mul)
                nc.vector.tensor_add(dst[cg][:, off:off + cw],
                                     t1[:, 0:cw], t2[:, 0:cw])

        for off, cw in QCH:
            proj_rope(qT, wq, wqr, qb, rqb, xTq0, xTq1, ctq, stq, off, cw)
        for off, cw in KCH:
            proj_rope(kT, wk, wkr,
                      [0.0, 0.0], [0.0, 0.0], xT0, xT1, ctk, stk, off, cw)

        # v projection (natural layout [n, c] in 33-wide head blocks; the
        # 33rd column stays 1.0 from the memset) + bias via K=1 matmul
        for t in range(NT):
            ps = scp.tile([128, 2, 512], F32, tag="sc")
            nc.tensor.matmul(ps[:, 0, :C], xT0[:, bass.ts(t, 128)], wv[0][:],
                             start=True, stop=False)
            nc.tensor.matmul(ps[:, 0, :C], xT1[:, bass.ts(t, 128)], wv[1][:],
                             start=False, stop=False)
            nc.tensor.matmul(ps[:, 0, :C], ones[0:1, :], vb[:],
                             start=False, stop=True)
            vdst = vsb[:, t * NH * VW:(t + 1) * NH * VW]
            vdst = vdst.rearrange("p (h c) -> p h c", c=VW)
            psrc = ps[:, 0, 0:C].rearrange("p (h c) -> p h c", c=32)
            nc.vector.tensor_copy(vdst[:, :, 0:32], psrc[:])

        # ---- phase 2+3: attention + output projection --------------------
        for qoff, cw in QCH:
            oT0 = outpool.tile([128, 512], F16, tag="o0")
            oT1 = outpool.tile([128, 512], F16, tag="o1")
            scw = 8 if "scores" in ablate else cw
            ecw = 8 if "exp" in ablate else cw
            acw = 8 if "avsum" in ablate else cw
            for g in range(NG):
                av = avp.tile([128, 512], F32, tag="av")
                for t in range(NT):
                    sc = scp.tile([128, 2, 512], F32, tag="sc")
                    for gi in range(2):
                        h = 2 * g + gi
                        cg, hh = h // 4, h % 4
                        nc.tensor.matmul(
                            sc[:, gi, :scw],
                            kT[cg][bass.ts(hh, 32), bass.ts(t, 128)],
                            qT[cg][bass.ts(hh, 32), qoff:qoff + scw],
                            start=True, stop=True, tile_position=(32 * hh, 0))
                    pt = ptpool.tile([128, 2, 512], F16, tag="pt")
                    nc.scalar.activation(pt[:, 0:2, 0:ecw],
                                         sc[:, 0:2, 0:ecw], AF.Exp)
                    for gi in range(2):
                        h = 2 * g + gi
                        nc.tensor.matmul(
                            av[64 * gi:64 * gi + VW, 0:acw],
                            vsb[:, (t * NH + h) * VW:(t * NH + h + 1) * VW],
                            pt[:, gi, 0:acw],
                            start=(t == 0), stop=(t == NT - 1),
                            skip_group_check=True)
                # finalize: recip of the fused sums rows (32 and 96),
                # broadcast via K=1 matmuls, then normalize
                rsb = rpool.tile([128, 512], F16, tag="rs")
                with nc.allow_low_precision("fp16 softmax scale rows"):
                    for gi in range(2):
                        r = 64 * gi + 32
                        nc.vector.reciprocal(rsb[r:r + 1, 0:cw],
                                             av[r:r + 1, 0:cw])
                rf = scp.tile([128, 2, 512], F32, tag="sc")
                for gi in range(2):
                    r = 64 * gi + 32
                    nc.tensor.matmul(rf[bass.ts(gi, 64), 0, 0:cw][0:32, :],
                                     ones[r:r + 1, 0:32],
                                     rsb[r:r + 1, 0:cw],
                                     start=True, stop=True,
                                     tile_position=(r, 64 * gi),
                                     skip_group_check=True)
                rfsb = rpool.tile([128, 512], F32, tag="rfsb")
                for gi in range(2):
                    nc.vector.tensor_copy(rfsb[64 * gi:64 * gi + 32, 0:cw],
                                          rf[64 * gi:64 * gi + 32, 0, 0:cw])
                dst = oT0 if g < 2 else oT1
                d0 = 64 * (g % 2)
                for gi in range(2):
                    nc.vector.tensor_mul(dst[d0 + 32 * gi:d0 + 32 * gi + 32, 0:cw],
                                         av[64 * gi:64 * gi + 32, 0:cw],
                                         rfsb[64 * gi:64 * gi + 32, 0:cw])
            # output projection for this chunk
            for s in range(cw // 128):
                yps = scp.tile([128, 2, 512], F32, tag="sc")
                nc.tensor.matmul(yps[:, 0, :C], oT0[:, bass.ts(s, 128)],
                                 wo[0][:], start=True, stop=False)
                nc.tensor.matmul(yps[:, 0, :C], oT1[:, bass.ts(s, 128)],
                                 wo[1][:], start=False, stop=True)
                ysb = ypool.tile([128, C], F32, tag="y")
                nc.vector.tensor_add(ysb[:], yps[:, 0, 0:C], bob[:])
                nc.sync.dma_start(io["y"][qoff + 128 * s: qoff + 128 * (s + 1), :],
                                  ysb[:])

        if R > 1:
            loop_ctx.__exit__(None, None, None)


def build_nc(R=1):
    nc = bacc.Bacc("TRN2", target_bir_lowering=False, debug=False,
                   enable_asserts=True, num_devices=8)
    io = {}
    for name, shape, dt in IN_SPECS:
        io[name] = nc.dram_tensor(name, shape, dt, kind="ExternalInput").ap()
    io["y"] = nc.dram_tensor("y", [NQ, C], F32, kind="ExternalOutput").ap()

    with tile.TileContext(nc) as tc:
        emit(tc, io, R=R)
    nc.compile()
    return nc


def host_inputs(x, Wq, q_bias, Wk, Wv, v_bias, Wo, bo):
    """Build the per-core input maps (host-side sharding + layout prep)."""
    xf = np.ascontiguousarray(x.reshape(B, N, C))

    inv_freq = 1.0 / (ROPE_BASE ** (np.arange(0, HD, 2, dtype=np.float64) / HD))
    pos = np.arange(N, dtype=np.float64)
    ang = pos[:, None] * inv_freq[None, :]          # [N, 16]
    cos_t, sin_t = np.cos(ang), np.sin(ang)         # [N, 16]
    # channel c -> within-head index jj = c % 32, freq f = jj % 16
    jj = np.arange(C) % HD
    f = jj % D2
    CT = cos_t[:, f].T                              # [C, N] float64
    ST = sin_t[:, f].T

    # signed rotate-half permutation RM [C, C]: partner = RM @ q
    RM = np.zeros((C, C), dtype=np.float64)
    for p in range(C):
        j = p % HD
        if j < D2:
            RM[p, p + D2] = -1.0                    # partner[p] = -q[p+16]
        else:
            RM[p, p - D2] = 1.0                     # partner[p] = +q[p-16]

    Wq64, Wk64 = Wq.astype(np.float64), Wk.astype(np.float64)
    Wqr = RM @ Wq64                                 # rotated projections
    Wkr = RM @ Wk64
    rqb = RM @ q_bias.astype(np.float64)

    f16 = lambda a: np.ascontiguousarray(a, dtype=np.float16)
    f32 = lambda a: np.ascontiguousarray(a, dtype=np.float32)

    common = {
        "wqT": f16(Wq64.T), "wqrT": f16(Wqr.T),
        "wkT": f16(Wk64.T), "wkrT": f16(Wkr.T),
        "wvT": f16(Wv.T), "woT": f16(Wo.T),
        "qb": f32(q_bias[:, None]), "rqb": f32(rqb[:, None]),
        "vb": f16(v_bias[None, :]),
        "ones": np.ones((128, 128), dtype=np.float16),
        "bob": f32(np.broadcast_to(bo, (128, C))),
        "CTK": f16(CT), "STK": f16(ST),
    }
    in_maps = []
    for core in range(8):
        b, qhalf = core // 2, core % 2
        qoff = qhalf * NQ
        xT = xf[b].T
        m = dict(common)
        m["xT"] = f16(xT)
        m["xTq"] = f16(xT[:, qoff:qoff + NQ])
        m["CTQ"] = f16(CT[:, qoff:qoff + NQ] * SCALE)
        m["STQ"] = f16(ST[:, qoff:qoff + NQ] * SCALE)
        in_maps.append(m)
    return in_maps


_NC_CACHE = {}


def get_nc(R=1):
    if R not in _NC_CACHE:
        _NC_CACHE[R] = build_nc(R)
    return _NC_CACHE[R]


def kernel(**inputs):
    inputs = {k: np.asarray(v, dtype=np.float32) for k, v in inputs.items()}
    in_maps = host_inputs(**inputs)
    nc = get_nc()
    res = run_bass_kernel_spmd(nc, in_maps, core_ids=list(range(8)))
    out = np.empty((B, N, C), dtype=np.float32)
    for core in range(8):
        b, qhalf = core // 2, core % 2
        qoff = qhalf * NQ
        out[b, qoff:qoff + NQ, :] = res.results[core]["y"]
    return out.reshape(B, HH, WW, C)



# revision 2
# speedup vs baseline: 1.2839x; 1.2839x over previous
"""Self-contained Trainium2 Bass kernel for nn_Attention_37125697306831.

Multi-head attention block: B=4, H=W=48 (N=2304), C=256, 8 heads, head_dim=32,
RoPE (rotate-half), softmax attention, separate Q/K/V projections (K without
bias), output projection with bias.

Sharding: 8 cores = (batch b in 0..3) x (query half in 0..1). Each core:
  - computes Q for its 1152 queries (all heads), K/V for all 2304 keys of its
    batch, attention + output projection for its 1152 query rows.
  - no collectives; output rows are disjoint across cores.

On-chip layouts:
  - xT [ci, n], qT/kT [c, n] (head dim on partitions), V natural [n, c].
  - scores computed transposed S.T[m keys, n queries] via row-packed K=32
    fp16 matmuls (tile_position), exp PSUM->SBUF split across ScalarE
    (exact Exp) and DVE (Schraudolph fp16 bit-trick: int16(x*A+B) bitcast),
    A@V as col-packed fp16 matmuls contracting over keys (K=128), softmax
    sums via a ones-column in V, normalization via per-head K=1 broadcast
    matmuls + DVE multiply, output projection consumes normalized out.T as
    lhsT giving y [n, co] for contiguous DMA out.

Pipelining: scores are emitted two key-tiles ahead of exp/A@V so the PE
never stalls on the activation engines; V-bias is folded into the output
bias on the host (rows of softmax sum to 1), removing its matmul.

All matmul operands are fp16 (PE full rate; PSUM accumulation is fp32);
elementwise math (RoPE, exp on ScalarE, reciprocal, bias adds) stays fp32.
"""

import numpy as np
from contextlib import ExitStack

import concourse.bass as bass
import concourse.tile as tile
from concourse import bacc, mybir
from concourse.bass_utils import run_bass_kernel_spmd

F32 = mybir.dt.float32
F16 = mybir.dt.float16
I16 = mybir.dt.int16
AF = mybir.ActivationFunctionType

B, HH, WW, C = 4, 48, 48, 256
N = HH * WW            # 2304 keys per batch
NQ = N // 2            # 1152 queries per core
NH, HD, D2 = 8, 32, 16
NT = N // 128          # 18 key m-tiles
ROPE_BASE = 10000.0
SCALE = HD ** -0.5

QCH = [(0, 512), (512, 512), (1024, 128)]                       # query chunks
KCH = [(0, 512), (512, 512), (1024, 512), (1536, 512), (2048, 256)]
NG = 4                  # 4 groups of 2 heads
VW = 33                 # V columns per head incl. the ones column

# Schraudolph fp16-bits exp: exp(x) ~= bitcast_f16(int16(x*EA + EB)).
# EB centered geometrically: 15360 - 1024*log2(sqrt(1.0861)) -> max rel 4.2%.
EA = 1477.3197218702985   # 2^10 / ln 2
EB = 15299.427
# key-tiles whose exp runs on DVE (rest on ScalarE)
DVE_T = frozenset(t for t in range(NT) if t % 3 == 2)

# DRAM input dtypes: fp16 for matmul operands, fp32 for DVE-side constants
IN_SPECS = [
    ("xT", [C, N], F16), ("xTq", [C, NQ], F16),
    ("wqT", [C, C], F16), ("wqrT", [C, C], F16),
    ("wkT", [C, C], F16), ("wkrT", [C, C], F16),
    ("wvT", [C, C], F16), ("woT", [C, C], F16),
    ("qb", [C, 1], F32), ("rqb", [C, 1], F32),
    ("ones", [128, 128], F16),
    ("bob", [128, C], F32),
    ("CTQ", [C, NQ], F16), ("STQ", [C, NQ], F16),
    ("CTK", [C, N], F16), ("STK", [C, N], F16),
]

mul = mybir.AluOpType.mult
add_op = mybir.AluOpType.add


def emit(tc, io, R=1):
    nc = tc.nc
    ctx = ExitStack()
    with ctx:
        consts = ctx.enter_context(tc.tile_pool(name="consts", bufs=1))
        sb = ctx.enter_context(tc.tile_pool(name="sb", bufs=1))
        tmp = ctx.enter_context(tc.tile_pool(name="tmp", bufs=4))
        ptpool = ctx.enter_context(tc.tile_pool(name="pt", bufs=4))
        outpool = ctx.enter_context(tc.tile_pool(name="outT", bufs=2))
        ypool = ctx.enter_context(tc.tile_pool(name="y", bufs=3))
        rpool = ctx.enter_context(tc.tile_pool(name="recip", bufs=2))
        # PSUM: scores 3x2 banks + av 2x1 = 8 banks; rf/y borrow scp slots
        scp = ctx.enter_context(tc.tile_pool(name="scp", bufs=3, space="PSUM"))
        avp = ctx.enter_context(tc.tile_pool(name="avp", bufs=2, space="PSUM"))

        dtypes = {name: dt for name, _, dt in IN_SPECS}

        def load(name, shape):
            t = consts.tile(shape, dtypes[name], tag=name)
            nc.sync.dma_start(t[:], io[name][:])
            return t

        # ---- constant loads ----------------------------------------------
        xT0 = consts.tile([128, N], F16, tag="xT0")
        nc.sync.dma_start(xT0[:], io["xT"][0:128, :])
        xT1 = consts.tile([128, N], F16, tag="xT1")
        nc.sync.dma_start(xT1[:], io["xT"][128:256, :])
        xTq0 = consts.tile([128, NQ], F16, tag="xTq0")
        nc.sync.dma_start(xTq0[:], io["xTq"][0:128, :])
        xTq1 = consts.tile([128, NQ], F16, tag="xTq1")
        nc.sync.dma_start(xTq1[:], io["xTq"][128:256, :])

        def load2(name, free):
            a = consts.tile([128, free], dtypes[name], tag=name + "0")
            nc.sync.dma_start(a[:], io[name][0:128, :])
            b_ = consts.tile([128, free], dtypes[name], tag=name + "1")
            nc.sync.dma_start(b_[:], io[name][128:256, :])
            return [a, b_]

        wq = load2("wqT", C)
        wqr = load2("wqrT", C)
        wk = load2("wkT", C)
        wkr = load2("wkrT", C)
        wv = load2("wvT", C)
        wo = load2("woT", C)
        qb = load2("qb", 1)
        rqb = load2("rqb", 1)
        ones = load("ones", [128, 128])
        bob = load("bob", [128, C])
        ctq = load2("CTQ", NQ)
        stq = load2("STQ", NQ)
        ctk = load2("CTK", N)
        stk = load2("STK", N)

        if R > 1:
            loop_ctx = tc.For_i(0, R, 1)
            loop_ctx.__enter__()

        # ---- phase 1: projections + RoPE ---------------------------------
        qT = [sb.tile([128, NQ], F16, tag=f"qT{i}", name=f"qT{i}") for i in range(2)]
        kT = [sb.tile([128, N], F16, tag=f"kT{i}", name=f"kT{i}") for i in range(2)]
        vsb = sb.tile([128, NT * NH * VW], F16, tag="v")
        nc.gpsimd.memset(vsb[:], 1.0)

        def proj_rope(dst, w, wr, bias, rbias, xa, xb, ct, st, off, cw):
            # dst[:, off:off+cw] = (w.T x + b) * ct + (wr.T x + rb) * st
            for cg in range(2):
                ps = scp.tile([128, 2, 512], F32, tag="sc")
                nc.tensor.matmul(ps[:, 0, :cw], w[0][:, bass.ts(cg, 128)],
                                 xa[:, off:off + cw], start=True, stop=False)
                nc.tensor.matmul(ps[:, 0, :cw], w[1][:, bass.ts(cg, 128)],
                                 xb[:, off:off + cw], start=False, stop=True)
                nc.tensor.matmul(ps[:, 1, :cw], wr[0][:, bass.ts(cg, 128)],
                                 xa[:, off:off + cw], start=True, stop=False)
                nc.tensor.matmul(ps[:, 1, :cw], wr[1][:, bass.ts(cg, 128)],
                                 xb[:, off:off + cw], start=False, stop=True)
                b0 = bias[cg] if isinstance(bias[cg], float) else bias[cg][:]
                b1 = rbias[cg] if isinstance(rbias[cg], float) else rbias[cg][:]
                t1 = tmp.tile([128, 512], F32, tag="t1")
                nc.vector.scalar_tensor_tensor(
                    t1[:, 0:cw], ps[:, 0, 0:cw], b0,
                    ct[cg][:, off:off + cw], op0=add_op, op1=mul)
                t2 = tmp.tile([128, 512], F32, tag="t2")
                nc.vector.scalar_tensor_tensor(
                    t2[:, 0:cw], ps[:, 1, 0:cw], b1,
                    st[cg][:, off:off + cw], op0=add_op, op1=mul)
                nc.gpsimd.tensor_add(dst[cg][:, off:off + cw],
                                     t1[:, 0:cw], t2[:, 0:cw])

        for off, cw in QCH:
            proj_rope(qT, wq, wqr, qb, rqb, xTq0, xTq1, ctq, stq, off, cw)
        for off, cw in KCH:
            proj_rope(kT, wk, wkr,
                      [0.0, 0.0], [0.0, 0.0], xT0, xT1, ctk, stk, off, cw)

        # v projection (natural layout [n, c] in 33-wide head blocks; the
        # 33rd column stays 1.0 from the memset); bias folded into bob.
        for t in range(NT):
            ps = scp.tile([128, 2, 512], F32, tag="sc")
            nc.tensor.matmul(ps[:, 0, :C], xT0[:, bass.ts(t, 128)], wv[0][:],
                             start=True, stop=False)
            nc.tensor.matmul(ps[:, 0, :C], xT1[:, bass.ts(t, 128)], wv[1][:],
                             start=False, stop=True)
            vdst = vsb[:, t * NH * VW:(t + 1) * NH * VW]
            vdst = vdst.rearrange("p (h c) -> p h c", c=VW)
            psrc = ps[:, 0, 0:C].rearrange("p (h c) -> p h c", c=32)
            nc.scalar.copy(vdst[:, :, 0:32], psrc[:])

        # ---- phase 2+3: attention + output projection --------------------
        for qoff, cw in QCH:
            oT0 = outpool.tile([128, 512], F16, tag="o0")
            oT1 = outpool.tile([128, 512], F16, tag="o1")
            for g in range(NG):
                av = avp.tile([128, 512], F32, tag="av")

                def scores(t):
                    sc = scp.tile([128, 2, 512], F32, tag="sc")
                    for gi in range(2):
                        h = 2 * g + gi
                        cg, hh = h // 4, h % 4
                        nc.tensor.matmul(
                            sc[:, gi, :cw],
                            kT[cg][bass.ts(hh, 32), bass.ts(t, 128)],
                            qT[cg][bass.ts(hh, 32), qoff:qoff + cw],
                            start=True, stop=True, tile_position=(32 * hh, 0))
                    return sc

                def exp_av(t, sc):
                    pt = ptpool.tile([128, 2, 512], F16, tag="pt")
                    if t in DVE_T:
                        nc.vector.tensor_scalar(
                            pt[:, 0:2, 0:cw].bitcast(I16), sc[:, 0:2, 0:cw],
                            EA, EB, op0=mul, op1=add_op)
                    else:
                        nc.scalar.activation(pt[:, 0:2, 0:cw],
                                             sc[:, 0:2, 0:cw], AF.Exp)
                    for gi in range(2):
                        h = 2 * g + gi
                        nc.tensor.matmul(
                            av[64 * gi:64 * gi + VW, 0:cw],
                            vsb[:, (t * NH + h) * VW:(t * NH + h + 1) * VW],
                            pt[:, gi, 0:cw],
                            start=(t == 0), stop=(t == NT - 1),
                            skip_group_check=True)

                pend = [scores(0), scores(1)]
                for t in range(NT):
                    if t + 2 < NT:
                        pend.append(scores(t + 2))
                    exp_av(t, pend.pop(0))

                # finalize: recip of the fused sums rows (32 and 96),
                # broadcast via K=1 matmuls, then normalize
                rsb = rpool.tile([128, 512], F16, tag="rs")
                with nc.allow_low_precision("fp16 softmax scale rows"):
                    for gi in range(2):
                        r = 64 * gi + 32
                        nc.vector.reciprocal(rsb[r:r + 1, 0:cw],
                                             av[r:r + 1, 0:cw])
                rf = scp.tile([128, 2, 512], F32, tag="sc")
                for gi in range(2):
                    r = 64 * gi + 32
                    nc.tensor.matmul(rf[bass.ts(gi, 64), 0, 0:cw][0:32, :],
                                     ones[r:r + 1, 0:32],
                                     rsb[r:r + 1, 0:cw],
                                     start=True, stop=True,
                                     tile_position=(r, 64 * gi),
                                     skip_group_check=True)
                rfsb = rpool.tile([128, 512], F32, tag="rfsb")
                for gi in range(2):
                    nc.vector.tensor_copy(rfsb[64 * gi:64 * gi + 32, 0:cw],
                                          rf[64 * gi:64 * gi + 32, 0, 0:cw])
                dst = oT0 if g < 2 else oT1
                d0 = 64 * (g % 2)
                for gi in range(2):
                    nc.vector.tensor_mul(dst[d0 + 32 * gi:d0 + 32 * gi + 32, 0:cw],
                                         av[64 * gi:64 * gi + 32, 0:cw],
                                         rfsb[64 * gi:64 * gi + 32, 0:cw])
            # output projection for this chunk
            for s in range(cw // 128):
                yps = scp.tile([128, 2, 512], F32, tag="sc")
                nc.tensor.matmul(yps[:, 0, :C], oT0[:, bass.ts(s, 128)],
                                 wo[0][:], start=True, stop=False)
                nc.tensor.matmul(yps[:, 0, :C], oT1[:, bass.ts(s, 128)],
                                 wo[1][:], start=False, stop=True)
                ysb = ypool.tile([128, C], F32, tag="y")
                nc.vector.tensor_add(ysb[:], yps[:, 0, 0:C], bob[:])
                nc.sync.dma_start(io["y"][qoff + 128 * s: qoff + 128 * (s + 1), :],
                                  ysb[:])

        if R > 1:
            loop_ctx.__exit__(None, None, None)


def build_nc(R=1):
    nc = bacc.Bacc("TRN2", target_bir_lowering=False, debug=False,
                   enable_asserts=True, num_devices=8)
    io = {}
    for name, shape, dt in IN_SPECS:
        io[name] = nc.dram_tensor(name, shape, dt, kind="ExternalInput").ap()
    io["y"] = nc.dram_tensor("y", [NQ, C], F32, kind="ExternalOutput").ap()

    with tile.TileContext(nc) as tc:
        emit(tc, io, R=R)
    nc.compile()
    return nc


def host_inputs(x, Wq, q_bias, Wk, Wv, v_bias, Wo, bo):
    """Build the per-core input maps (host-side sharding + layout prep)."""
    xf = np.ascontiguousarray(x.reshape(B, N, C))

    inv_freq = 1.0 / (ROPE_BASE ** (np.arange(0, HD, 2, dtype=np.float64) / HD))
    pos = np.arange(N, dtype=np.float64)
    ang = pos[:, None] * inv_freq[None, :]          # [N, 16]
    cos_t, sin_t = np.cos(ang), np.sin(ang)         # [N, 16]
    # channel c -> within-head index jj = c % 32, freq f = jj % 16
    jj = np.arange(C) % HD
    f = jj % D2
    CT = cos_t[:, f].T                              # [C, N] float64
    ST = sin_t[:, f].T

    # signed rotate-half permutation RM [C, C]: partner = RM @ q
    RM = np.zeros((C, C), dtype=np.float64)
    for p in range(C):
        j = p % HD
        if j < D2:
            RM[p, p + D2] = -1.0                    # partner[p] = -q[p+16]
        else:
            RM[p, p - D2] = 1.0                     # partner[p] = +q[p-16]

    Wq64, Wk64 = Wq.astype(np.float64), Wk.astype(np.float64)
    Wqr = RM @ Wq64                                 # rotated projections
    Wkr = RM @ Wk64
    rqb = RM @ q_bias.astype(np.float64)

    # v_bias folded into the output bias: softmax rows sum to 1, so
    # attn @ (V + 1 b_v^T) = attn @ V + 1 b_v^T  ->  y += Wo @ b_v.
    bo2 = bo.astype(np.float64) + Wo.astype(np.float64) @ v_bias.astype(np.float64)

    f16 = lambda a: np.ascontiguousarray(a, dtype=np.float16)
    f32 = lambda a: np.ascontiguousarray(a, dtype=np.float32)

    common = {
        "wqT": f16(Wq64.T), "wqrT": f16(Wqr.T),
        "wkT": f16(Wk64.T), "wkrT": f16(Wkr.T),
        "wvT": f16(Wv.T), "woT": f16(Wo.T),
        "qb": f32(q_bias[:, None]), "rqb": f32(rqb[:, None]),
        "ones": np.ones((128, 128), dtype=np.float16),
        "bob": f32(np.broadcast_to(bo2, (128, C))),
        "CTK": f16(CT), "STK": f16(ST),
    }
    in_maps = []
    for core in range(8):
        b, qhalf = core // 2, core % 2
        qoff = qhalf * NQ
        xT = xf[b].T
        m = dict(common)
        m["xT"] = f16(xT)
        m["xTq"] = f16(xT[:, qoff:qoff + NQ])
        m["CTQ"] = f16(CT[:, qoff:qoff + NQ] * SCALE)
        m["STQ"] = f16(ST[:, qoff:qoff + NQ] * SCALE)
        in_maps.append(m)
    return in_maps


_NC_CACHE = {}


def get_nc(R=1):
    if R not in _NC_CACHE:
        _NC_CACHE[R] = build_nc(R)
    return _NC_CACHE[R]


def kernel(**inputs):
    inputs = {k: np.asarray(v, dtype=np.float32) for k, v in inputs.items()}
    in_maps = host_inputs(**inputs)
    nc = get_nc()
    res = run_bass_kernel_spmd(nc, in_maps, core_ids=list(range(8)))
    out = np.empty((B, N, C), dtype=np.float32)
    for core in range(8):
        b, qhalf = core // 2, core % 2
        qoff = qhalf * NQ
        out[b, qoff:qoff + NQ, :] = res.results[core]["y"]
    return out.reshape(B, HH, WW, C)
